# revision 15
# baseline (speedup 1.0000x reference)
"""MixHop GNN (nn_MixHopNetwork_75299366633514) on 8 TRN2 NeuronCores.

Horner-restructured formulation (mathematically identical to the reference):
  r_j = relu(X @ Wu_j + bu_j)                       j = 0,1,2
  C_i = Wb_i @ fc_w[i*H:(i+1)*H]                    [600, 40]  (host)
  logits = sum_{i,k} (A^{i+k} r_k) @ C_i^{(k)} + const
         = s0 + A(s1 + A(s2 + A(s3 + A s4)))        (Horner, width 40)
  where [s0..s4] = R @ Chat,  R = [r0|r1|r2],  Chat block-Toeplitz from C_i,
  const = sum_i bb_i @ fc_i + fc_b (folded into s0 via a constant-1 unit).
  out = log_softmax(logits)

All four spmm hops operate on width-40 (padded 128) operands instead of
512/256-wide ones; the dense bottom layers and final fc collapse into one
[768 x 320] matmul fused with D1.

Sharding: nodes contiguous across 8 cores (12800 padded rows/core); edges by
destination; AllGather of the 128-wide hop operand per hop, split into 4
row-quarters for overlap.  Gathers use prepare_only descriptors + trigger_dma
so the Pool engine is not serialized on DMA completion.
"""

import numpy as np
import ml_dtypes

bf16 = ml_dtypes.bfloat16

SB = 64    # padded s-block width (real CL=40)
NS = 5     # number of s blocks (A^0..A^4)
HB = 128   # hop operand width (gather elem), bf16 -> 256B rows


class Cfg:
    def __init__(self, N, F, H, HP, CL, rpc_raw, rpc, qrows, deg_scale=None):
        self.NC = 8
        self.N = N
        self.F = F
        self.H = H
        self.HP = HP
        self.CL = CL
        self.RPC_RAW = rpc_raw
        self.RPC = rpc
        self.QROWS = qrows
        self.NBLK = rpc // 128
        self.NWIN = 4
        self.WIN_ROWS = self.NC * qrows
        assert rpc == 4 * qrows and qrows % 128 == 0
        assert self.WIN_ROWS <= 32768
        self.NH = HP // 128
        self.NJ = 3 * self.NH
        assert HP % 128 == 0 and H < HP  # need at least one pad unit for const


FULL = Cfg(N=100000, F=512, H=200, HP=256, CL=40,
           rpc_raw=12500, rpc=12800, qrows=3200)


# ---------------------------------------------------------------- host side

def preprocess(cfg, features, adj_row, adj_col, adj_val, Wu, bu, Wb, bb, fc_w, fc_b):
    NC, RPCR, RPC, QROWS = cfg.NC, cfg.RPC_RAW, cfg.RPC, cfg.QROWS
    NBLK, NWIN = cfg.NBLK, cfg.NWIN
    H, HP, F, CL = cfg.H, cfg.HP, cfg.F, cfg.CL
    NH, NJ = cfg.NH, cfg.NJ

    row = adj_row.astype(np.int64)
    col = adj_col.astype(np.int64)
    val = adj_val.astype(np.float32)

    core = row // RPCR
    dl = row - core * RPCR
    b = dl >> 7
    doff = dl & 127
    cs = col // RPCR
    ls = col - cs * RPCR
    w = ls // QROWS
    idx = cs * QROWS + (ls - w * QROWS)

    # (w, b)-major slot layout: for fixed window, consecutive blocks are
    # contiguous so one gather call can span a group of destination blocks.
    key = (core * NWIN + w) * NBLK + b
    counts = np.bincount(key, minlength=NC * NWIN * NBLK).reshape(NC, NWIN, NBLK)
    Cs = np.maximum(1, -(-counts.max(axis=0) // 128))  # [NWIN, NBLK]
    seg_off = np.zeros((NWIN, NBLK), np.int64)
    flat = Cs.reshape(-1)
    seg_off.reshape(-1)[1:] = np.cumsum(flat)[:-1]
    TOT = int(flat.sum()) * 128

    order = np.lexsort((b, w, core))
    so_r, so_w, so_b = core[order], w[order], b[order]
    gkey = (so_r * NWIN + so_w) * NBLK + so_b
    grp_start = np.zeros(len(gkey), np.int64)
    new_grp = np.ones(len(gkey), bool)
    new_grp[1:] = gkey[1:] != gkey[:-1]
    starts_idx = np.nonzero(new_grp)[0]
    grp_start[starts_idx] = starts_idx
    grp_start = np.maximum.accumulate(grp_start)
    cum = np.arange(len(gkey)) - grp_start
    slot = seg_off[so_w, so_b] * 128 + cum

    idx_pad = np.zeros((NC, TOT), np.int16)
    doff_pad = np.zeros((NC, TOT), np.int16)
    val_pad = np.zeros((NC, TOT), np.float32)
    idx_pad[so_r, slot] = idx[order].astype(np.int16)
    doff_pad[so_r, slot] = doff[order].astype(np.int16)
    val_pad[so_r, slot] = val[order]

    idx_dev = np.zeros((NC, 128, TOT // 16), np.int16)
    blk = np.transpose(idx_pad.reshape(NC, TOT // 16, 16), (0, 2, 1))
    for k in range(8):
        idx_dev[:, 16 * k:16 * (k + 1), :] = blk
    doff_dev = np.ascontiguousarray(
        np.transpose(doff_pad.reshape(NC, TOT // 128, 128), (0, 2, 1))
    ).astype(bf16)
    val_dev = np.ascontiguousarray(
        np.transpose(val_pad.reshape(NC, TOT // 128, 128), (0, 2, 1))
    ).astype(bf16)

    # features: per-core [F, RPC] bf16
    featsT = np.zeros((NC, F, RPC), bf16)
    for c in range(NC):
        featsT[c, :, :RPCR] = features[c * RPCR:(c + 1) * RPCR].T.astype(bf16)

    # dense weights
    Wu_p = np.zeros((3, F, HP), bf16)
    Wu_p[:, :, :H] = Wu.astype(bf16)
    buT = np.zeros((128, NJ), np.float32)
    for i in range(3):
        for h in range(NH):
            j = i * NH + h
            lo, hi = h * 128, min((h + 1) * 128, H)
            if hi > lo:
                buT[:hi - lo, j] = bu[i, lo:hi]
    # constant-1 unit: first pad position of r0 (row H of block 0)
    buT[H % 128, H // 128] = 1.0

    Cm = [Wb[i].astype(np.float64) @ fc_w[i * H:(i + 1) * H].astype(np.float64)
          for i in range(3)]
    const = sum(bb[i].astype(np.float64) @ fc_w[i * H:(i + 1) * H].astype(np.float64)
                for i in range(3)) + fc_b.astype(np.float64)
    ChatP = np.zeros((3 * HP, NS * SB), np.float64)
    for i in range(3):
        for k in range(3):
            s = i + k
            ChatP[k * HP:k * HP + H, s * SB:s * SB + CL] += Cm[i][k * H:(k + 1) * H]
    ChatP[H, 0:CL] += const  # routed through the constant-1 unit (r0 pad row H)
    chat_dev = np.zeros((128, NJ, NS * SB), bf16)
    for j in range(NJ):
        chat_dev[:, j, :] = ChatP[j * 128:(j + 1) * 128, :].astype(bf16)

    iota = np.broadcast_to(np.arange(128, dtype=np.float32), (128, 128)).astype(bf16)

    in_maps = []
    for c in range(NC):
        in_maps.append({
            "featsT": featsT[c],
            "meta_idx": idx_dev[c],
            "meta_doff": doff_dev[c],
            "meta_val": val_dev[c],
            "Wu_p": Wu_p,
            "buT": buT,
            "chat": chat_dev,
            "iota": np.asarray(iota),
        })
    segs2 = [[(int(seg_off[ww, bb_]), int(Cs[ww, bb_])) for bb_ in range(NBLK)]
             for ww in range(NWIN)]
    return in_maps, segs2, TOT


# -------------------------------------------------------------- device side

def build_nc(cfg, segs2, TOT, MAXC=7, G=2):
    import concourse.bacc as bacc
    import concourse.mybir as mybir
    import concourse.tile as tile

    dt = mybir.dt
    NC, RPC, QROWS = cfg.NC, cfg.RPC, cfg.QROWS
    NBLK, NWIN, WIN_ROWS = cfg.NBLK, cfg.NWIN, cfg.WIN_ROWS
    F, HP, CL = cfg.F, cfg.HP, cfg.CL
    NH, NJ = cfg.NH, cfg.NJ
    KF = F // 128
    SW = NS * SB
    ALL = list(range(NC))
    MG2 = 256
    NG2 = RPC // MG2
    MPG = MG2 // 128

    nc = bacc.Bacc("TRN2", target_bir_lowering=False, debug=False, num_devices=NC)

    featsT = nc.dram_tensor("featsT", [F, RPC], dt.bfloat16, kind="ExternalInput")
    meta_idx = nc.dram_tensor("meta_idx", [128, TOT // 16], dt.int16, kind="ExternalInput")
    meta_doff = nc.dram_tensor("meta_doff", [128, TOT // 128], dt.bfloat16, kind="ExternalInput")
    meta_val = nc.dram_tensor("meta_val", [128, TOT // 128], dt.bfloat16, kind="ExternalInput")
    Wu_p = nc.dram_tensor("Wu_p", [3, F, HP], dt.bfloat16, kind="ExternalInput")
    buT_in = nc.dram_tensor("buT", [128, NJ], dt.float32, kind="ExternalInput")
    chat_in = nc.dram_tensor("chat", [128, NJ, SW], dt.bfloat16, kind="ExternalInput")
    iota_in = nc.dram_tensor("iota", [128, 128], dt.bfloat16, kind="ExternalInput")
    y_out = nc.dram_tensor("y_out", [RPC, CL], dt.float32, kind="ExternalOutput")

    with tile.TileContext(nc) as tc:
        with (
            tc.tile_pool(name="const", bufs=1) as cpool,
            tc.tile_pool(name="dram", bufs=1, space="DRAM") as dram,
        ):
            iota_t = cpool.tile([128, 128], dt.bfloat16)
            nc.sync.dma_start(iota_t[:], iota_in[:])
            idx_t = cpool.tile([128, TOT // 16], dt.int16)
            nc.sync.dma_start(idx_t[:], meta_idx[:])
            doff_t = cpool.tile([128, TOT // 128], dt.bfloat16)
            nc.sync.dma_start(doff_t[:], meta_doff[:])
            val_t = cpool.tile([128, TOT // 128], dt.bfloat16)
            nc.sync.dma_start(val_t[:], meta_val[:])
            wu_t = cpool.tile([128, 3, KF, HP], dt.bfloat16)
            for i in range(3):
                for kc in range(KF):
                    nc.sync.dma_start(wu_t[:, i, kc, :], Wu_p[i, kc * 128:(kc + 1) * 128, :])
            but_t = cpool.tile([128, NJ], dt.float32)
            nc.sync.dma_start(but_t[:], buT_in[:])
            chat_t = cpool.tile([128, NJ, SW], dt.bfloat16)
            for j in range(NJ):
                nc.sync.dma_start(chat_t[:, j, :], chat_in[:, j, :])

            s_store = dram.tile([RPC, 4 * SB], dt.float32)
            agin = [[dram.tile([QROWS, HB], dt.bfloat16, name=f"agin{s}_{q}")
                     for q in range(4)] for s in range(4)]
            agbuf = [[dram.tile([WIN_ROWS, HB], dt.bfloat16, addr_space="Shared",
                                name=f"agbuf{s}_{q}") for q in range(4)]
                     for s in range(4)]

            # One sem per DMASW lane: Tile assigns gen_mode==1 preps to the 8
            # DMASW procs round-robin and computes per-lane wait thresholds;
            # the completion sem baked into each prep's descriptors must match
            # its lane or thresholds under/over-count.
            gsems = [nc.alloc_semaphore(f"gather_dma{i}") for i in range(8)]
            prep_counter = [0]
            sem_counts = [0] * 8

            def next_gsem():
                i = prep_counter[0] % 8
                prep_counter[0] += 1
                sem_counts[i] += 16
                return gsems[i], i

            def allgather(st):
                for q in range(4):
                    nc.gpsimd.collective_compute(
                        "AllGather", mybir.AluOpType.bypass,
                        replica_groups=[ALL],
                        ins=[agin[st][q][:].opt()],
                        outs=[agbuf[st][q][:].opt()],
                    )

            # ============ Phase A: r_j^T = relu(Wu_j^T X^T), S = R @ Chat
            # The hop SBUF pool stays open alongside Phase A's so their SBUF
            # regions are disjoint: the hop gathers are deferred (prepare_only)
            # writes, which must not land in reused Phase A buffers.
            with (
                tc.tile_pool(name="pa", bufs=2) as dp,
                tc.tile_pool(name="hops", bufs=2) as hp,
            ):
              with (
                  tc.tile_pool(name="paD", bufs=2, space="PSUM") as ppD,
                  tc.tile_pool(name="paS", bufs=2, space="PSUM") as ppS,
              ):
                for g in range(NG2):
                    xt = dp.tile([128, KF, MG2], dt.bfloat16, name="xt", tag="xt", bufs=2)
                    for kc in range(KF):
                        nc.sync.dma_start(
                            xt[:, kc, :],
                            featsT[kc * 128:(kc + 1) * 128, g * MG2:(g + 1) * MG2])
                    psD = ppD.tile([128, NJ, MG2], dt.float32, name="psD", tag="psD", bufs=2)
                    for j in range(NJ):
                        i, h = divmod(j, NH)
                        for kc in range(KF):
                            nc.tensor.matmul(
                                psD[:, j, :],
                                wu_t[:, i, kc, h * 128:(h + 1) * 128],
                                xt[:, kc, :],
                                start=(kc == 0), stop=(kc == KF - 1))
                    rtT = dp.tile([128, NJ, MG2], dt.bfloat16, name="rtT", tag="rtT", bufs=2)
                    for j in range(NJ):
                        nc.scalar.activation(rtT[:, j, :], psD[:, j, :],
                                             mybir.ActivationFunctionType.Relu,
                                             bias=but_t[:, j:j + 1])
                    for ml in range(MPG):
                        m = g * MPG + ml
                        psS = ppS.tile([128, SW], dt.float32, name="psS", tag="psS", bufs=2)
                        for j in range(NJ):
                            nc.tensor.matmul(
                                psS[:],
                                rtT[:, j, ml * 128:(ml + 1) * 128],
                                chat_t[:, j, :],
                                start=(j == 0), stop=(j == NJ - 1))
                        sst = dp.tile([128, 4 * SB], dt.float32, name="sst", tag="sst", bufs=3)
                        nc.scalar.activation(sst[:], psS[:, 0:4 * SB],
                                             mybir.ActivationFunctionType.Copy)
                        nc.sync.dma_start(s_store[m * 128:(m + 1) * 128, :], sst[:])
                        st4 = dp.tile([128, HB], dt.bfloat16, name="st4", tag="st4", bufs=3)
                        nc.vector.memset(st4[:, SB:HB], 0.0)
                        nc.scalar.activation(st4[:, 0:SB], psS[:, 4 * SB:5 * SB],
                                             mybir.ActivationFunctionType.Copy)
                        q, lr = divmod(m * 128, QROWS)
                        nc.sync.dma_start(agin[0][q][lr:lr + 128, :], st4[:])

              # ============ hop stages (shared SBUF/PSUM pools across stages)
              with tc.tile_pool(name="hopp", bufs=4, space="PSUM") as hpp:
                def hop(st, s_idx, out_stage):
                    for gb0 in range(0, NBLK, G):
                        blocks = list(range(gb0, min(gb0 + G, NBLK)))
                        sumC = sum(segs2[w_][b][1] for w_ in range(NWIN) for b in blocks)
                        gt = hp.tile([128, sumC, HB], dt.bfloat16, name="gt", tag="gt", bufs=2)
                        oh = hp.tile([128, sumC, 128], dt.bfloat16, name="oh", tag="oh", bufs=2)
                        wbase = {}
                        cur = 0
                        gwaits = {}
                        for w_ in range(NWIN):
                            off0 = segs2[w_][blocks[0]][0]
                            L = sum(segs2[w_][b][1] for b in blocks)
                            wbase[w_] = (cur, off0)
                            done = 0
                            while done < L:
                                Cg = min(L - done, MAXC)
                                nc.gpsimd.dma_gather(
                                    gt[:, cur + done:cur + done + Cg, :],
                                    agbuf[st][w_][:],
                                    idx_t[:, (off0 + done) * 8:(off0 + done + Cg) * 8],
                                    num_idxs=Cg * 128,
                                    num_idxs_reg=Cg * 128,
                                    elem_size=HB)
                                done += Cg
                            nc.vector.tensor_tensor(
                                oh[:, cur:cur + L, :],
                                iota_t[:].unsqueeze(1).broadcast_to([128, L, 128]),
                                doff_t[:, off0:off0 + L].unsqueeze(2).broadcast_to([128, L, 128]),
                                mybir.AluOpType.is_equal)
                            nc.vector.tensor_tensor(
                                oh[:, cur:cur + L, :], oh[:, cur:cur + L, :],
                                val_t[:, off0:off0 + L].unsqueeze(2).broadcast_to([128, L, 128]),
                                mybir.AluOpType.mult)
                            cur += L
                        for b in blocks:
                            ps = hpp.tile([128, HB], dt.float32, name="hps", tag="hps", bufs=4)
                            plist = []
                            for w_ in range(NWIN):
                                base, off0 = wbase[w_]
                                boff = base + (segs2[w_][b][0] - off0)
                                plist += [boff + c for c in range(segs2[w_][b][1])]
                            for k, p in enumerate(plist):
                                nc.tensor.matmul(ps[:], oh[:, p, :], gt[:, p, :],
                                                 start=(k == 0), stop=(k == len(plist) - 1))
                            q, lr = divmod(b * 128, QROWS)
                            sld = hp.tile([128, SB], dt.float32, name="sld", tag="sld", bufs=3)
                            nc.sync.dma_start(
                                sld[:], s_store[b * 128:(b + 1) * 128, s_idx * SB:(s_idx + 1) * SB])
                            if out_stage is not None:
                                ot = hp.tile([128, HB], dt.bfloat16, name="ot", tag="ot", bufs=4)
                                nc.vector.memset(ot[:, SB:HB], 0.0)
                                nc.vector.tensor_add(ot[:, 0:SB], ps[:, 0:SB], sld[:])
                                nc.sync.dma_start(agin[out_stage][q][lr:lr + 128, :], ot[:])
                            else:
                                lg = hp.tile([128, SB], dt.float32, name="lg", tag="lg", bufs=3)
                                nc.vector.tensor_add(lg[:], ps[:, 0:SB], sld[:])
                                mx = hp.tile([128, 1], dt.float32, name="mx", tag="mx", bufs=3)
                                nc.vector.tensor_reduce(mx[:], lg[:, 0:CL],
                                                        mybir.AxisListType.X,
                                                        mybir.AluOpType.max, negate=True)
                                ex = hp.tile([128, CL], dt.float32, name="ex", tag="ex", bufs=3)
                                sm = hp.tile([128, 1], dt.float32, name="sm", tag="sm", bufs=3)
                                nc.scalar.activation(ex[:], lg[:, 0:CL],
                                                     mybir.ActivationFunctionType.Exp,
                                                     bias=mx[:], accum_out=sm[:])
                                ln = hp.tile([128, 1], dt.float32, name="ln", tag="ln", bufs=3)
                                nc.scalar.activation(ln[:], sm[:],
                                                     mybir.ActivationFunctionType.Ln)
                                fo = hp.tile([128, CL], dt.float32, name="fo", tag="fo", bufs=3)
                                nc.vector.tensor_scalar(fo[:], lg[:, 0:CL], mx[:], ln[:],
                                                        mybir.AluOpType.add,
                                                        mybir.AluOpType.subtract)
                                nc.sync.dma_start(y_out[b * 128:(b + 1) * 128, :], fo[:])

                allgather(0)
                hop(0, s_idx=3, out_stage=1)
                allgather(1)
                hop(1, s_idx=2, out_stage=2)
                allgather(2)
                hop(2, s_idx=1, out_stage=3)
                allgather(3)
                hop(3, s_idx=0, out_stage=None)

    nc.compile()
    return nc


# ------------------------------------------------------------------- driver

def run(cfg, inputs, trace=False, mode="hw", MAXC=7, G=2):
    in_maps, segs2, TOT = preprocess(cfg, **inputs)
    nc = build_nc(cfg, segs2, TOT, MAXC=MAXC, G=G)
    outs = np.zeros((cfg.N, cfg.CL), np.float32)
    if mode == "sim":
        from concourse.bass_interp import MultiCoreSim
        sim = MultiCoreSim(nc, num_cores=cfg.NC, trace=False)
        for c, core in enumerate(sim.cores.values()):
            for k, v in in_maps[c].items():
                core.tensor(k)[:] = v
        sim.simulate()
        for c, core in enumerate(sim.cores.values()):
            outs[c * cfg.RPC_RAW:(c + 1) * cfg.RPC_RAW] = \
                np.asarray(core.tensor("y_out"))[:cfg.RPC_RAW]
        return outs, None
    from concourse import bass_utils
    res = bass_utils.run_bass_kernel_spmd(
        nc, in_maps, core_ids=list(range(cfg.NC)), trace=trace)
    for c in range(cfg.NC):
        outs[c * cfg.RPC_RAW:(c + 1) * cfg.RPC_RAW] = \
            res.results[c]["y_out"][:cfg.RPC_RAW]
    return outs, res


def kernel(**inputs):
    inputs = {k: np.asarray(v) for k, v in inputs.items()}
    out, _ = run(FULL, inputs, trace=False)
    return out
